# revision 2
# baseline (speedup 1.0000x reference)
"""Trainium2 Bass kernel v2 for EnhancedMessageLayer (GNN message passing).

Strategy (8 NeuronCores, SPMD, no collectives):
  * Nodes split into 8 slices of 6250; each core owns the edges whose dst
    node falls in its slice.  Per core: 50 tiles of 125 dst nodes.
  * x.T (bf16, [128, 50000], rotated so the core's own nodes sit first)
    stays resident in SBUF; per-edge source rows are fetched with gpsimd
    ap_gather at node-PAIR granularity (d=2, idx = src>>1).  Edges are
    bucketed by src parity so every 128-edge chunk reads a uniform
    stride-2 lhsT slice (d-slot 0 = even src, 1 = odd src).
  * Per chunk: h1[e,f] = xsrcT.T @ W1a + onehot_ne.T @ yext, where yext
    rows 0:125 hold x_tile @ W1t + b1 and rows 125:128 hold W1e, and the
    one-hot's rows 125:128 carry edge_attr.T — so edge_attr costs no
    extra matmul.  relu -> aggT[f,n] += A.T @ onehot_en (PSUM, one
    accumulation group per 4-tile group bank).
  * Both one-hot orientations are host-built bf16 and DMA'd.
  * Update phase batched over 4-tile groups, all matmuls bf16, biases as
    rank-1 matmuls, sigmoid/relu/copies on ACT (single act table, no
    table reloads), LayerNorm via bn_stats/bn_aggr + Newton rsqrt on DVE.
    Output written bf16; host casts to f32.
"""

import numpy as np
import ml_dtypes

P = 128
N_NODES = 50000
N_EDGES = 640000
H = 128
NC = 8
PC = N_NODES // NC        # 6250
TN = 125                  # dst nodes per tile
NT = PC // TN             # 50 tiles per core
NEP = N_NODES // 2        # 25000 node pairs
GS = 4                    # tiles per update group
NG = (NT + GS - 1) // GS  # 13 groups (12x4 + 1x2)

_kernel_cache = {}
bfloat16 = ml_dtypes.bfloat16


def _groups():
    return [list(range(g * GS, min(g * GS + GS, NT))) for g in range(NG)]


# --------------------------------------------------------------------------
# Host-side preprocessing
# --------------------------------------------------------------------------

def _prep(x, edge_index, edge_attr):
    src = np.asarray(edge_index[0], dtype=np.int64)
    dst = np.asarray(edge_index[1], dtype=np.int64)
    ea = np.asarray(edge_attr, dtype=np.float32)

    core = dst // PC
    tile = (dst % PC) // TN
    dslot = dst % TN
    parity = src & 1

    kE = np.zeros((NC, NT), np.int64)
    kO = np.zeros((NC, NT), np.int64)
    per_ct = {}
    for c in range(NC):
        cm = core == c
        for t in range(NT):
            idxs = np.nonzero(cm & (tile == t))[0]
            pe = idxs[parity[idxs] == 0]
            po = idxs[parity[idxs] == 1]
            kE[c, t] = len(pe)
            kO[c, t] = len(po)
            per_ct[c, t] = (pe, po)

    cE = np.maximum(np.ceil(kE / P).astype(int).max(axis=0), 1)
    cO = np.maximum(np.ceil(kO / P).astype(int).max(axis=0), 1)
    CPT = int((cE + cO).max())

    maps = []
    for c in range(NC):
        idx_a = np.zeros((NG, P, GS * CPT * 8), np.int16)
        ohne = np.zeros((NT, P, 2 * CPT * P), bfloat16)
        deg = np.zeros((1, NT * P), bfloat16)
        x4 = np.zeros((NG, TN, GS * P), bfloat16)
        for t in range(NT):
            et = int(cE[t] + cO[t]) * P
            pe, po = per_ct[c, t]
            nE = int(cE[t]) * P
            slots = np.full(et, -1, np.int64)
            slots[: len(pe)] = pe
            slots[nE : nE + len(po)] = po
            valid = slots >= 0
            sv = slots[valid]

            # gather indices: rotated source node >> 1
            src_rot = (src[sv] - c * PC) % N_NODES
            idx_flat = np.zeros(et, np.int64)
            idx_flat[valid] = src_rot >> 1
            blk = idx_flat.reshape(-1, 16).T.astype(np.int16)
            g, i = divmod(t, GS)
            for k in range(8):
                idx_a[g, 16 * k : 16 * (k + 1),
                      i * CPT * 8 : i * CPT * 8 + et // 16] = blk

            ds = np.full(et, -1, np.int64)
            ds[valid] = dslot[sv]
            oh = ds[:, None] == np.arange(TN)[None, :]        # [et, 125]
            ohne[t, :TN, :et] = oh.T.astype(bfloat16)
            eat = np.zeros((et, 3), np.float32)
            eat[valid] = ea[sv]
            ohne[t, TN : TN + 3, :et] = eat.T.astype(bfloat16)
            ohb = np.zeros((et, P), np.float32)
            ohb[:, :TN] = oh
            ohne[t, :, CPT * P : CPT * P + et] = (
                ohb.reshape(-1, P, P).transpose(1, 0, 2).reshape(P, et)
                .astype(bfloat16)
            )
            dg = np.zeros(P, np.float32)
            np.add.at(dg, ds[valid], 1.0)
            deg[0, t * P : (t + 1) * P] = dg.astype(bfloat16)

        xs = x[c * PC : (c + 1) * PC]
        for t in range(NT):
            g, i = divmod(t, GS)
            x4[g, :, i * P : (i + 1) * P] = (
                xs[t * TN : (t + 1) * TN].astype(bfloat16)
            )
        maps.append(dict(idx=idx_a, ohne=ohne, deg=deg, x4=x4))
    return maps, tuple(int(v) for v in cE), tuple(int(v) for v in cO), CPT


def _weight_map(kw):
    bf = bfloat16
    f32 = np.float32
    W1 = np.asarray(kw["W_msg1"], f32)
    w1e3x = np.zeros((P, H), np.float32)
    w1e3x[TN : TN + 3] = W1[2 * H : 2 * H + 3]
    return dict(
        w1a=W1[:H].astype(bf),
        w1t32=np.ascontiguousarray(W1[H : 2 * H]),
        w1e3x=w1e3x.astype(bf),
        identb=np.eye(P, dtype=f32).astype(bf),
        b1b=np.tile(np.asarray(kw["b_msg1"], f32)[None, :], (P, 1)),
        w2=np.asarray(kw["W_msg2"], f32).copy(),
        b2row=np.asarray(kw["b_msg2"], f32)[None, :].astype(bf),
        wgx=np.asarray(kw["W_gate"], f32)[:H].astype(bf),
        wga=np.ascontiguousarray(np.asarray(kw["W_gate"], f32)[H:]),
        bgrow=np.asarray(kw["b_gate"], f32)[None, :].astype(bf),
        wu1x=np.asarray(kw["W_upd1"], f32)[:H].astype(bf),
        wu1a=np.ascontiguousarray(np.asarray(kw["W_upd1"], f32)[H:]),
        bu1col=np.asarray(kw["b_upd1"], f32)[:, None].copy(),
        wu2=np.asarray(kw["W_upd2"], f32).astype(bf),
        bu2row=np.asarray(kw["b_upd2"], f32)[None, :].astype(bf),
        gammab=np.tile(np.asarray(kw["ln_gamma"], f32)[None, :], (P, 1)).astype(bf),
        betab=np.tile(np.asarray(kw["ln_beta"], f32)[None, :], (P, 1)).astype(bf),
    )


# --------------------------------------------------------------------------
# Bass kernel builder
# --------------------------------------------------------------------------

def _build(cE, cO, CPT):
    import concourse.bacc as bacc
    import concourse.tile as tile
    from concourse import mybir

    f32 = mybir.dt.float32
    bf16 = mybir.dt.bfloat16
    i16 = mybir.dt.int16
    i32 = mybir.dt.int32
    Alu = mybir.AluOpType
    Act = mybir.ActivationFunctionType
    groups = _groups()
    cpt = [cE[t] + cO[t] for t in range(NT)]

    nc = bacc.Bacc("TRN2", target_bir_lowering=False, debug=False,
                   num_devices=NC)

    xT_d = nc.dram_tensor("xT", [P, N_NODES], bf16, kind="ExternalInput")
    idx_d = nc.dram_tensor("idx", [NG, P, GS * CPT * 8], i16,
                           kind="ExternalInput")
    ohne_d = nc.dram_tensor("ohne", [NT, P, 2 * CPT * P], bf16,
                            kind="ExternalInput")
    deg_d = nc.dram_tensor("deg", [1, NT * P], bf16, kind="ExternalInput")
    x4_d = nc.dram_tensor("x4", [NG, TN, GS * P], bf16, kind="ExternalInput")
    wnames = [
        ("w1a", [H, H], bf16), ("w1t32", [H, H], f32),
        ("w1e3x", [P, H], bf16), ("identb", [P, P], bf16),
        ("b1b", [P, H], f32), ("w2", [H, H], f32), ("b2row", [1, H], bf16),
        ("wgx", [H, H], bf16), ("wga", [H, H], f32), ("bgrow", [1, H], bf16),
        ("wu1x", [H, H], bf16), ("wu1a", [H, H], f32),
        ("bu1col", [H, 1], f32), ("wu2", [H, H], bf16),
        ("bu2row", [1, H], bf16), ("gammab", [P, H], bf16),
        ("betab", [P, H], bf16),
    ]
    wd = {n: nc.dram_tensor(n, s, d, kind="ExternalInput") for n, s, d in wnames}
    out_d = nc.dram_tensor("out", [TN, NT * H], bf16, kind="ExternalOutput")

    with tile.TileContext(nc) as tc:
        with (
            tc.tile_pool(name="const", bufs=1) as cpool,
            tc.tile_pool(name="sg", bufs=3) as sgp,
            tc.tile_pool(name="ohp", bufs=3) as ohp,
            tc.tile_pool(name="meta", bufs=2) as meta,
            tc.tile_pool(name="yx", bufs=3) as yx,
            tc.tile_pool(name="ab", bufs=3) as abp,
            tc.tile_pool(name="upd", bufs=2) as updp,
            tc.tile_pool(name="agt", bufs=2) as agtp,
            tc.tile_pool(name="ln", bufs=2) as lnp,
            tc.tile_pool(name="ps_h1", bufs=2, space="PSUM") as ps_h1,
            tc.tile_pool(name="ps_agg", bufs=2, space="PSUM") as ps_agg,
            tc.tile_pool(name="ps_y", bufs=2, space="PSUM") as ps_y,
            tc.tile_pool(name="ps_u", bufs=2, space="PSUM") as ps_u,
        ):
            W = {}
            for n, s, d in wnames:
                W[n] = cpool.tile(s, d, tag=n, name=f"w_{n}")
                nc.scalar.dma_start(out=W[n][:], in_=wd[n][:])
            ones1 = cpool.tile([1, P], bf16)
            nc.vector.memset(ones1[:], 1.0)
            deg_sb = cpool.tile([1, NT * P], bf16)
            nc.scalar.dma_start(out=deg_sb[:], in_=deg_d[:])
            xT_sb = cpool.tile([P, N_NODES], bf16)
            nc.sync.dma_start(out=xT_sb[:], in_=xT_d[:])

            rctr = [0]  # relu engine alternator
            gstate = {}

            def emit_edge(g, tiles):
                ng = len(tiles)
                idx4 = meta.tile([P, GS * CPT * 8], i16, tag="idx",
                                 name=f"idx_{g}")
                nc.scalar.dma_start(out=idx4[:], in_=idx_d[g, :, :])
                x4 = meta.tile([TN, GS * P], bf16, tag="x4", name=f"x4_{g}")
                nc.scalar.dma_start(out=x4[:], in_=x4_d[g, :, :])

                aggb = ps_agg.tile([P, 512], f32, tag="agg", name=f"agg_{g}")
                xTt = {}
                for i, t in enumerate(tiles):
                    ct = cpt[t]
                    et = ct * P
                    oh = ohp.tile([P, 2 * CPT * P], bf16, tag="oh",
                                  name=f"oh_{t}")
                    nc.sync.dma_start(out=oh[:, :et], in_=ohne_d[t, :, :et])
                    nc.scalar.dma_start(
                        out=oh[:, CPT * P : CPT * P + et],
                        in_=ohne_d[t, :, CPT * P : CPT * P + et],
                    )
                    sg = sgp.tile([P, CPT * P, 2], bf16, tag="sg",
                                  name=f"sg_{t}")
                    nc.gpsimd.ap_gather(
                        sg[:, :et, :], xT_sb[:],
                        idx4[:, i * CPT * 8 : i * CPT * 8 + et // 16],
                        channels=P, num_elems=NEP, d=2, num_idxs=et,
                    )
                    xTt[i] = xT_sb[:, t * TN : (t + 1) * TN]
                    # fp32 y = x_tile @ W1t + b1 (bf16 y-rounding would be
                    # amplified by node degree); split hi/lo for the two
                    # bf16 target matmuls.
                    xTf = yx.tile([H, TN], f32, tag="xtf", name=f"xtf_{t}")
                    nc.scalar.activation(out=xTf[:], in_=xTt[i],
                                         func=Act.Copy)
                    y_ps = ps_y.tile([P, P], f32, tag="y", name=f"y_{t}")
                    nc.tensor.matmul(out=y_ps[:TN, :], lhsT=xTf[:],
                                     rhs=W["w1t32"][:], start=True, stop=True)
                    yf = yx.tile([TN, H], f32, tag="yf", name=f"yf_{t}")
                    nc.vector.tensor_tensor(out=yf[:], in0=y_ps[:TN, :],
                                            in1=W["b1b"][:TN, :], op=Alu.add)
                    # yext rows 125:128 carry W1e (static); rows 0:125 get
                    # bf16(yf).  Full-tile copy first keeps partition starts
                    # legal (0/32/64/96 only) and the race detector happy.
                    yext = yx.tile([P, H], bf16, tag="yext", name=f"yext_{t}")
                    nc.scalar.activation(out=yext[:], in_=W["w1e3x"][:],
                                         func=Act.Copy)
                    nc.vector.tensor_copy(out=yext[:TN, :], in_=yf[:])
                    ylo = yx.tile([TN, H], bf16, tag="ylo", name=f"ylo_{t}")
                    nc.vector.tensor_tensor(out=ylo[:], in0=yf[:],
                                            in1=yext[:TN, :], op=Alu.subtract)

                    for c0 in range(0, ct, 4):
                        nq = min(4, ct - c0)
                        h1 = ps_h1.tile([P, 512], f32, tag="h1",
                                        name=f"h1_{t}_{c0}")
                        for q in range(nq):
                            ci = c0 + q
                            par = 0 if ci < cE[t] else 1
                            nc.tensor.matmul(
                                out=h1[:, q * P : (q + 1) * P],
                                lhsT=sg[:, ci * P : (ci + 1) * P, par],
                                rhs=W["w1a"][:],
                                start=(q == 0), stop=False,
                            )
                            nc.tensor.matmul(
                                out=h1[:, q * P : (q + 1) * P],
                                lhsT=oh[:, ci * P : (ci + 1) * P],
                                rhs=yext[:],
                                start=False, stop=False,
                            )
                            nc.tensor.matmul(
                                out=h1[:, q * P : (q + 1) * P],
                                lhsT=oh[:TN, ci * P : (ci + 1) * P],
                                rhs=ylo[:],
                                start=False, stop=(q == nq - 1),
                            )
                        A = abp.tile([P, 512], bf16, tag="A",
                                     name=f"A_{t}_{c0}")
                        if rctr[0] % 2 == 0:
                            nc.vector.tensor_single_scalar(
                                out=A[:, : nq * P], in_=h1[:, : nq * P],
                                scalar=0.0, op=Alu.max,
                            )
                        else:
                            nc.scalar.activation(
                                out=A[:, : nq * P], in_=h1[:, : nq * P],
                                func=Act.Relu,
                            )
                        rctr[0] += 1
                        for q in range(nq):
                            ci = c0 + q
                            nc.tensor.matmul(
                                out=aggb[:, i * P : i * P + TN],
                                lhsT=A[:, q * P : (q + 1) * P],
                                rhs=oh[:, CPT * P + ci * P :
                                       CPT * P + ci * P + TN],
                                start=(i == 0 and ci == 0),
                                stop=(i == ng - 1 and ci == ct - 1),
                            )

                gstate[g] = (x4, xTt, aggb)

            def emit_update(g, tiles):
                ng = len(tiles)
                x4, xTt, aggb = gstate.pop(g)
                agg2 = ps_u.tile([P, 512], f32, tag="u", name=f"ag2_{g}")
                for i, t in enumerate(tiles):
                    aT = agtp.tile([P, TN], f32, tag=f"aT{i}", name=f"aT_{t}")
                    nc.scalar.activation(out=aT[:],
                                         in_=aggb[:, i * P : i * P + TN],
                                         func=Act.Copy)
                    nc.tensor.matmul(out=agg2[:, i * TN : (i + 1) * TN],
                                     lhsT=W["w2"][:], rhs=aT[:],
                                     start=(i == 0), stop=False)
                    nc.tensor.matmul(
                        out=agg2[:, i * TN : (i + 1) * TN],
                        lhsT=W["b2row"][:],
                        rhs=deg_sb[:, t * P : t * P + TN],
                        start=False, stop=(i == ng - 1),
                    )
                a2T = updp.tile([P, GS * TN], f32, tag="a2T", name=f"a2T_{g}")
                nc.scalar.activation(out=a2T[:, : ng * TN],
                                     in_=agg2[:, : ng * TN], func=Act.Copy)

                gate_ps = ps_u.tile([TN, GS * P], f32, tag="u",
                                    name=f"gate_{g}")
                for i, t in enumerate(tiles):
                    cs = slice(i * P, (i + 1) * P)
                    nc.tensor.matmul(out=gate_ps[:, cs], lhsT=xTt[i],
                                     rhs=W["wgx"][:], start=(i == 0),
                                     stop=False)
                    nc.tensor.matmul(out=gate_ps[:, cs],
                                     lhsT=a2T[:, i * TN : (i + 1) * TN],
                                     rhs=W["wga"][:], start=False, stop=False)
                    nc.tensor.matmul(out=gate_ps[:, cs],
                                     lhsT=ones1[:, :TN], rhs=W["bgrow"][:],
                                     start=False, stop=(i == ng - 1))
                gate = updp.tile([TN, GS * P], bf16, tag="gate",
                                 name=f"gate_{g}")
                nc.scalar.activation(out=gate[:, : ng * P],
                                     in_=gate_ps[:, : ng * P],
                                     func=Act.Sigmoid)

                u1_ps = ps_u.tile([P, 512], f32, tag="u", name=f"u1_{g}")
                for i, t in enumerate(tiles):
                    cs = slice(i * TN, (i + 1) * TN)
                    nc.tensor.matmul(out=u1_ps[:, cs], lhsT=W["wu1x"][:],
                                     rhs=xTt[i], start=(i == 0), stop=False)
                    nc.tensor.matmul(out=u1_ps[:, cs], lhsT=W["wu1a"][:],
                                     rhs=a2T[:, cs], start=False,
                                     stop=(i == ng - 1))
                UT = updp.tile([P, GS * TN], bf16, tag="UT", name=f"UT_{g}")
                nc.scalar.activation(out=UT[:, : ng * TN],
                                     in_=u1_ps[:, : ng * TN], func=Act.Relu,
                                     bias=W["bu1col"][:, :1])

                upd_ps = ps_u.tile([TN, GS * P], f32, tag="u",
                                   name=f"upd_{g}")
                for i, t in enumerate(tiles):
                    cs = slice(i * P, (i + 1) * P)
                    nc.tensor.matmul(out=upd_ps[:, cs],
                                     lhsT=UT[:, i * TN : (i + 1) * TN],
                                     rhs=W["wu2"][:], start=(i == 0),
                                     stop=False)
                    nc.tensor.matmul(out=upd_ps[:, cs],
                                     lhsT=ones1[:, :TN], rhs=W["bu2row"][:],
                                     start=False, stop=(i == ng - 1))
                updb = updp.tile([TN, GS * P], bf16, tag="updb",
                                 name=f"updb_{g}")
                nc.scalar.activation(out=updb[:, : ng * P],
                                     in_=upd_ps[:, : ng * P], func=Act.Copy)

                d1 = lnp.tile([TN, GS * P], bf16, tag="d1", name=f"d1_{g}")
                nc.vector.tensor_tensor(out=d1[:, : ng * P],
                                        in0=updb[:, : ng * P],
                                        in1=x4[:, : ng * P], op=Alu.subtract)
                d2 = lnp.tile([TN, GS * P], bf16, tag="d2", name=f"d2_{g}")
                nc.vector.tensor_tensor(out=d2[:, : ng * P],
                                        in0=d1[:, : ng * P],
                                        in1=gate[:, : ng * P], op=Alu.mult)
                out0 = lnp.tile([TN, GS * P], bf16, tag="o0", name=f"o0_{g}")
                nc.vector.tensor_tensor(out=out0[:, : ng * P],
                                        in0=d2[:, : ng * P],
                                        in1=x4[:, : ng * P], op=Alu.add)

                mv = lnp.tile([TN, 2 * GS], f32, tag="mv", name=f"mv_{g}")
                for i, t in enumerate(tiles):
                    st6 = lnp.tile([TN, 6], f32, tag=f"st{i}", name=f"st_{t}")
                    nc.vector.bn_stats(st6[:], out0[:, i * P : (i + 1) * P])
                    nc.vector.bn_aggr(mv[:, 2 * i : 2 * i + 2], st6[:])
                # max(x,0)+eps keeps the unused mean lanes positive/finite
                nw = 2 * ng
                vpe = lnp.tile([TN, 2 * GS], f32, tag="vpe", name=f"vpe_{g}")
                nc.vector.tensor_scalar(out=vpe[:, :nw], in0=mv[:, :nw],
                                        scalar1=0.0, scalar2=1e-5,
                                        op0=Alu.max, op1=Alu.add)
                rv = lnp.tile([TN, 2 * GS], f32, tag="rv", name=f"rv_{g}")
                nc.vector.tensor_scalar(out=rv[:, :nw].bitcast(i32),
                                        in0=vpe[:, :nw].bitcast(i32),
                                        scalar1=1, scalar2=None,
                                        op0=Alu.arith_shift_right)
                nc.vector.tensor_scalar(out=rv[:, :nw].bitcast(i32),
                                        in0=rv[:, :nw].bitcast(i32),
                                        scalar1=0x5F3759DF, scalar2=-1,
                                        op0=Alu.subtract, op1=Alu.mult)
                for it in range(2):
                    t1 = lnp.tile([TN, 2 * GS], f32, tag=f"nt{it}",
                                  name=f"nt_{g}_{it}")
                    nc.vector.tensor_tensor(out=t1[:, :nw], in0=rv[:, :nw],
                                            in1=rv[:, :nw], op=Alu.mult)
                    nc.vector.tensor_tensor(out=t1[:, :nw], in0=t1[:, :nw],
                                            in1=vpe[:, :nw], op=Alu.mult)
                    nc.vector.tensor_scalar(out=t1[:, :nw], in0=t1[:, :nw],
                                            scalar1=-0.5, scalar2=1.5,
                                            op0=Alu.mult, op1=Alu.add)
                    nc.vector.tensor_tensor(out=rv[:, :nw], in0=rv[:, :nw],
                                            in1=t1[:, :nw], op=Alu.mult)

                outg = lnp.tile([TN, GS * P], bf16, tag="outg",
                                name=f"outg_{g}")
                for i, t in enumerate(tiles):
                    cs = slice(i * P, (i + 1) * P)
                    an = lnp.tile([TN, P], bf16, tag=f"an{i}", name=f"an_{t}")
                    nc.vector.tensor_scalar(
                        out=an[:], in0=out0[:, cs],
                        scalar1=mv[:, 2 * i : 2 * i + 1],
                        scalar2=rv[:, 2 * i + 1 : 2 * i + 2],
                        op0=Alu.subtract, op1=Alu.mult,
                    )
                    g1 = lnp.tile([TN, P], bf16, tag=f"g1{i}", name=f"g1_{t}")
                    nc.vector.tensor_tensor(out=g1[:], in0=an[:],
                                            in1=W["gammab"][:TN, :],
                                            op=Alu.mult)
                    nc.vector.tensor_tensor(out=outg[:, cs], in0=g1[:],
                                            in1=W["betab"][:TN, :],
                                            op=Alu.add)
                nc.sync.dma_start(
                    out=out_d[:, g * GS * H : g * GS * H + ng * H],
                    in_=outg[:, : ng * P],
                )

            for g, tiles in enumerate(groups):
                emit_edge(g, tiles)
                if g > 0:
                    emit_update(g - 1, groups[g - 1])
            emit_update(NG - 1, groups[NG - 1])

    nc.compile()
    return nc


# --------------------------------------------------------------------------
# Public entry point
# --------------------------------------------------------------------------

def build_in_maps(**inputs):
    x = np.asarray(inputs["x"], dtype=np.float32)
    maps, cE, cO, CPT = _prep(x, inputs["edge_index"], inputs["edge_attr"])
    wm = _weight_map(inputs)
    xT = np.ascontiguousarray(x.T)
    in_maps = []
    for c in range(NC):
        m = dict(maps[c])
        m["xT"] = np.ascontiguousarray(
            np.roll(xT, -c * PC, axis=1)
        ).astype(bfloat16)
        m.update(wm)
        in_maps.append(m)
    return (cE, cO, CPT), in_maps


def get_kernel(build_args):
    if build_args not in _kernel_cache:
        _kernel_cache[build_args] = _build(*build_args)
    return _kernel_cache[build_args]


def assemble(results):
    full = np.empty((N_NODES, H), np.float32)
    for c in range(NC):
        o = results[c].astype(np.float32)          # [TN, NT*H]
        full[c * PC : (c + 1) * PC] = (
            o.reshape(TN, NT, H).transpose(1, 0, 2).reshape(PC, H)
        )
    return full


def kernel(**inputs):
    import time
    from concourse.bass_utils import run_bass_kernel_spmd

    build_args, in_maps = build_in_maps(**inputs)
    nc = get_kernel(build_args)
    last_err = None
    for attempt in range(3):
        try:
            res = run_bass_kernel_spmd(nc, in_maps, list(range(NC)))
            outs = [res.results[c]["out"] for c in range(NC)]
            return assemble(outs)
        except Exception as e:
            last_err = e
            time.sleep(2.0)
    raise last_err
